# revision 30
# baseline (speedup 1.0000x reference)
"""Multi-head graph attention (GAT) on 8 TRN2 NeuronCores.

Reference computation (N=4096 nodes, F_in=512, H=8 heads, F_out=64):
    Wh   = einsum('nf,hfo->hno', features, W)
    src  = Wh @ a_src  (per head), dst = Wh @ a_dst
    e    = leaky_relu(src_i + dst_j, 0.2), masked by adjacency
    attn = softmax(e, axis=-1)
    h    = elu(attn @ Wh)  -> concat heads -> [N, H*F_out]

Sharding: head parallelism - core c owns head c entirely. Each core computes
its head's Wh, the full [N, N] masked softmax, and output columns
out[:, 64c:64c+64]; the host gather is a concatenate. No collectives.

Per-core algorithm ("keys on partitions", scores transposed):
    exp(prelu(src_i + dst_j)) = max(exp(z), exp(0.2 z))        [z = s_i + d_j]
                              = F1[j] * E2[i] * max(E4[i], C0[j])
with E4 = exp(0.8 src), E2 = exp(0.2 src), F1 = exp(dst), C0 = exp(-0.8 dst).
The E2[i] factor is constant per query column, so it scales softmax numerator
and denominator identically and CANCELS - dropped entirely. F1[j] rides the
contraction index, so it folds into the matmul weights (including the ones
column that accumulates the denominator):
    acc[o,i] = sum_j (Wh[j,o]*F1[j]) * pm[j,i]
    pm[j,i]  = adj[j,i] * max(E4[i], C0[j])
pm is ONE fused scalar_tensor_tensor op, (e4rep max C0) * adj, split across
Pool and DVE; ACT only computes the O(N) exp vectors and the epilogue.
Adjacency streams as fp8 ({0,1} exact) to halve HBM traffic. Normalization +
ELU run on transposed [P, 65] tiles at the end.
"""
import numpy as np
import ml_dtypes

import concourse.bass as bass
import concourse.bacc as bacc
import concourse.tile as tile
import concourse.mybir as mybir
from concourse.bass_utils import run_bass_kernel_spmd

FP32 = mybir.dt.float32
BF16 = mybir.dt.bfloat16
FP8 = mybir.dt.float8e4
AF = mybir.ActivationFunctionType
ALU = mybir.AluOpType
AX = mybir.AxisListType

P = 128          # SBUF partitions
N = 4096         # nodes
F = 512          # input features
H = 8            # heads
FO = 64          # out features per head
C = 8            # cores (1 head each)
JT = N // P      # key tiles = 32
FC = F // P      # feature chunks = 4
QC = N // 512    # query column chunks of 512 = 8
ALPHA = 0.2

# --- tuning knobs (HW-microbenched: DVE ts 4x=1.12us, DVE tt bf16 2x=2.21us,
# Pool tt=7.7us at gpsimd eff 0.42, DVE stt 1x=4.35us, ACT pass=3.5us) ---
# Every tile's max-job (v = e4rep max C0) is a 4x tensor_scalar on DVE.
# The mask-mult splits: MULT_DVE tiles do a 2x bf16 tensor_tensor on DVE
# (bf16 adjacency); the rest multiply on Pool (fp8 adjacency - Pool tt is
# dtype-agnostic, and gpsimd supports only tensor_tensor/copy on HW).
POOL_TILES = (0, 3, 6, 9, 12, 15, 18, 21, 24, 26)
MULT_DVE = tuple(j for j in range(32) if j not in POOL_TILES)
DVE_SLOT = {j: k for k, j in enumerate(MULT_DVE)}


def build_nc(iters=1, loop_n=None, upto=3):
    nc = bacc.Bacc("TRN2", target_bir_lowering=False, debug=False)

    d_ft = nc.dram_tensor("featT", [F, N], BF16, kind="ExternalInput")
    n_dve = len(MULT_DVE)
    # uint8 at the XLA boundary (fp8 unsupported there on TRN2); the DMA
    # bitcasts to fp8e4m3 - same bytes
    d_adj8 = nc.dram_tensor("adjT8", [N - n_dve * P, N], mybir.dt.uint8,
                            kind="ExternalInput")
    d_adjb = nc.dram_tensor("adjTb", [n_dve * P, N], BF16,
                            kind="ExternalInput")
    d_wh = nc.dram_tensor("Wh", [F, FO], BF16, kind="ExternalInput")
    d_ah = nc.dram_tensor("ah", [2, FO], BF16, kind="ExternalInput")
    d_id = nc.dram_tensor("ident", [P, P], FP32, kind="ExternalInput")
    d_out = nc.dram_tensor("out", [N, FO], FP32, kind="ExternalOutput")

    from contextlib import ExitStack, nullcontext

    with tile.TileContext(nc) as tc:
      with (tc.For_i(0, loop_n, 1) if loop_n else nullcontext()):
       for _it in range(iters):
        with ExitStack() as stk:
            keep = stk.enter_context(tc.tile_pool(name="keep", bufs=1))

            # ---- persistent tiles ----
            e4rep = keep.tile([P, N], BF16)        # exp(0.8 src) replicated
            wf1 = [keep.tile([P, FO + 1], BF16, name=f"wf1{j}", tag=f"wf1{j}")
                   for j in range(JT)]
            sdg = [keep.tile([P, 8], FP32, name=f"sdg{g}", tag=f"sdg{g}")
                   for g in range(4)]              # dst proj per group of 8
            f1g = [keep.tile([P, 8], FP32, name=f"f1g{g}", tag=f"f1g{g}")
                   for g in range(4)]              # exp(dst)
            c0g = [keep.tile([P, 8], FP32, name=f"c0g{g}", tag=f"c0g{g}")
                   for g in range(4)]              # exp(-0.8 dst)
            idn = keep.tile([P, P], FP32)
            ones1 = keep.tile([1, P], BF16)
            ar = keep.tile([1, 2 * FO], BF16)
            arep = keep.tile([P, 2 * FO], BF16)
            wt = keep.tile([P, 2 * FC], BF16)      # col 2c = src, 2c+1 = dst
            ht = keep.tile([FO + 1, N], FP32)      # evacuated accumulator

            nc.scalar.dma_start(idn[:], d_id[:])
            nc.scalar.dma_start(ar[:], d_ah.ap().rearrange("(x s) o -> x (s o)", x=1))
            nc.vector.memset(ones1[:], 1.0)

            with ExitStack() as ph1:
                sb1 = ph1.enter_context(tc.tile_pool(name="sb1", bufs=1))
                ps1 = ph1.enter_context(tc.tile_pool(name="ps1", bufs=2, space="PSUM"))

                ft = sb1.tile([P, FC * N], BF16)         # featT, 32KB/part
                whsd = sb1.tile([P, FC * (FO + 1)], BF16)  # [Wh_c | wt_dst_c]
                # small weights first so wt/psr/pwh aren't stuck behind the
                # big feature loads; ft halves split across the two hwdge
                # queues (h0 -> ACT, h1 -> SP) so both finish by ~13us
                nc.scalar.dma_start(
                    whsd[:].rearrange("p (c o) -> p c o", c=FC)[:, :, 0:FO],
                    d_wh.ap().rearrange("(c p) o -> p c o", p=P))
                for c_ in range(FC):
                    nc.scalar.dma_start(
                        ft[:, c_ * N: c_ * N + 2048],
                        d_ft[c_ * P:(c_ + 1) * P, 0:2048])
                for c_ in range(FC):
                    nc.sync.dma_start(
                        ft[:, c_ * N + 2048: c_ * N + 4096],
                        d_ft[c_ * P:(c_ + 1) * P, 2048:4096])

                # broadcast [a_src | a_dst] across partitions (k=1 matmul)
                ps_b = ps1.tile([P, 2 * FO], FP32, tag="bc", bufs=1)
                nc.tensor.matmul(ps_b[:], ones1[:], ar[:], start=True, stop=True)
                nc.vector.tensor_copy(arep[:], ps_b[:])

                # wtilde[f] = sum_o Wh[f, o] * a[o]  (src col -> wt, dst col ->
                # wt and whsd's 65th column per chunk)
                lp = stk.enter_context(
                    nc.allow_low_precision(reason="bf16 projection weights"))
                for c_ in range(FC):
                    tmp = sb1.tile([P, 2 * FO], BF16, tag="wtmp")
                    nc.vector.tensor_tensor(
                        tmp[:, 0:FO],
                        whsd[:, c_ * (FO + 1):c_ * (FO + 1) + FO],
                        arep[:, 0:FO], ALU.mult)
                    nc.vector.tensor_tensor(
                        tmp[:, FO:2 * FO],
                        whsd[:, c_ * (FO + 1):c_ * (FO + 1) + FO],
                        arep[:, FO:2 * FO], ALU.mult)
                    nc.vector.tensor_reduce(wt[:, 2 * c_:2 * c_ + 2],
                                            tmp[:].rearrange("p (s o) -> p s o", s=2),
                                            AX.X, ALU.add)
                    nc.vector.tensor_copy(whsd[:, c_ * (FO + 1) + FO:(c_ + 1) * (FO + 1)],
                                          wt[:, 2 * c_ + 1:2 * c_ + 2])

                # stage-major: all psr matmuls first, then all exps, then
                # all broadcasts - engine queues pipeline independently, so
                # the per-chunk PE<->ACT round-trips disappear.
                er4all = sb1.tile([1, N], BF16)
                psrs = {}

                def emit_psr(q):
                    psr = ps1.tile([1, 512], FP32, name=f"psr{q}",
                                   tag="psr", bufs=2)
                    for c_ in range(FC):
                        nc.tensor.matmul(psr[:], wt[:, 2 * c_:2 * c_ + 1],
                                         ft[:, c_ * N + q * 512:c_ * N + (q + 1) * 512],
                                         start=(c_ == 0), stop=(c_ == FC - 1))
                    psrs[q] = psr

                def emit_exp(q):
                    nc.scalar.activation(er4all[:, q * 512:(q + 1) * 512],
                                         psrs[q][:], AF.Exp, scale=0.8)

                def emit_bcast(q):
                    pb = ps1.tile([P, 512], FP32, tag="pb", bufs=2)
                    nc.tensor.matmul(pb[:], ones1[:],
                                     er4all[:, q * 512:(q + 1) * 512],
                                     start=True, stop=True)
                    nc.scalar.copy(e4rep[:, q * 512:(q + 1) * 512], pb[:])

                whd_tiles = {}

                def emit_group(g):
                    # 8 tiles: fused Wh+dst matmuls, then exp packs, then Wf1
                    for j in range(8 * g, 8 * g + 8):
                        pw = ps1.tile([P, FO + 1], FP32, name=f"pw{j}",
                                      tag="pw", bufs=3)
                        for c_ in range(FC):
                            nc.tensor.matmul(pw[:], ft[:, c_ * N + j * P: c_ * N + j * P + P],
                                             whsd[:, c_ * (FO + 1):(c_ + 1) * (FO + 1)],
                                             start=(c_ == 0), stop=(c_ == FC - 1))
                        whd = sb1.tile([P, FO], BF16, name=f"whd{j}",
                                       tag=f"whd{j % 16}")
                        nc.scalar.copy(whd[:], pw[:, 0:FO])
                        nc.scalar.copy(sdg[g][:, j % 8:j % 8 + 1],
                                       pw[:, FO:FO + 1])
                        whd_tiles[j] = whd
                    nc.scalar.activation(f1g[g][:], sdg[g][:], AF.Exp)
                    nc.scalar.activation(c0g[g][:], sdg[g][:], AF.Exp,
                                         scale=-0.8)
                    for j in range(8 * g, 8 * g + 8):
                        nc.vector.tensor_scalar(wf1[j][:, 0:FO], whd_tiles[j][:],
                                                f1g[g][:, j % 8:j % 8 + 1],
                                                None, ALU.mult)
                        nc.scalar.copy(wf1[j][:, FO:FO + 1],
                                       f1g[g][:, j % 8:j % 8 + 1])

                for q in range(QC):
                    emit_psr(q)
                for q in range(QC):
                    emit_exp(q)
                for q in range(QC):
                    emit_bcast(q)
                for g in range(4):
                    emit_group(g)

            if upto < 1:
                with ExitStack() as phx:
                    sbx = phx.enter_context(tc.tile_pool(name="sbx", bufs=1))
                    junk = sbx.tile([P, FO], FP32)
                    nc.vector.memset(junk[:], 0.0)
                    for i in range(JT):
                        nc.sync.dma_start(d_out[i * P:(i + 1) * P, :], junk[:])
                continue
            # ---- phase 2: fused masked-exp scores + V-matmul ----
            sb2 = stk.enter_context(tc.tile_pool(name="sb2", bufs=2))
            adjp = stk.enter_context(tc.tile_pool(name="adjp", bufs=4))
            with ExitStack() as ph2:
                acc_pool = ph2.enter_context(
                    tc.tile_pool(name="accps", bufs=1, space="PSUM"))
                acc = acc_pool.tile([FO + 1, N], FP32)   # all 8 banks

                pool_slot = 0
                for j in range(JT):
                    g, s = j // 8, j % 8
                    pm = sb2.tile([P, N], BF16, tag="pm", bufs=6)
                    v = sb2.tile([P, N], BF16, tag="v", bufs=4)
                    # pm = (e4rep max C0[j]) * adj
                    nc.vector.tensor_scalar(v[:], e4rep[:],
                                            c0g[g][:, s:s + 1], None, ALU.max)
                    if j in DVE_SLOT:
                        k = DVE_SLOT[j]
                        at = adjp.tile([P, N], BF16, tag="atb", bufs=4)
                        dq = nc.scalar if k % 2 == 0 else nc.sync
                        dq.dma_start(at[:], d_adjb[k * P:(k + 1) * P, :])
                        nc.vector.tensor_tensor(pm[:], at[:], v[:], ALU.mult)
                    else:
                        k = pool_slot
                        pool_slot += 1
                        at = adjp.tile([P, N], FP8, tag="at", bufs=4)
                        nc.sync.dma_start(at[:], d_adj8[k * P:(k + 1) * P, :].bitcast(FP8))
                        nc.gpsimd.tensor_tensor(pm[:], at[:], v[:], ALU.mult)
                    for q in range(QC):
                        nc.tensor.matmul(acc[:, q * 512:(q + 1) * 512], wf1[j][:],
                                         pm[:, q * 512:(q + 1) * 512],
                                         start=(j == 0), stop=(j == JT - 1))

                for q in range(4):
                    nc.scalar.copy(ht[:, q * 1024:(q + 1) * 1024],
                                   acc[:, q * 1024:(q + 1) * 1024])

            if upto < 2:
                with ExitStack() as phx:
                    sbx = phx.enter_context(tc.tile_pool(name="sbx", bufs=1))
                    junk = sbx.tile([P, FO], FP32)
                    nc.vector.memset(junk[:], 0.0)
                    for i in range(JT):
                        nc.sync.dma_start(d_out[i * P:(i + 1) * P, :], junk[:])
                continue
            # ---- epilogue: transpose, normalize, ELU ----
            # y = tp * rcol;  elu(y) = max(y, min(exp(y),1) - 1)
            # Stage-major emission: each engine streams 32 independent ops
            # per stage instead of ping-ponging cross-engine per tile
            # (in-order queues turn interleaved chains into serial
            # round-trips on HW). tp tiles pack 4 transposes per PSUM bank.
            with ExitStack() as ph3:
                ps3 = ph3.enter_context(tc.tile_pool(name="ps3", bufs=1, space="PSUM"))
                sb3 = ph3.enter_context(tc.tile_pool(name="sb3", bufs=1))
                tp4 = [ps3.tile([P, 4 * (FO + 1)], FP32, name=f"tp4{b}",
                                tag=f"tp4{b}") for b in range(8)]
                rcs = [sb3.tile([P, 1], FP32, name=f"rc{i}", tag=f"rc{i}")
                       for i in range(JT)]
                eys = [sb3.tile([P, FO], FP32, name=f"eyt{i}", tag=f"eyt{i}")
                       for i in range(JT)]
                ems = [sb3.tile([P, FO], FP32, name=f"emt{i}", tag=f"emt{i}")
                       for i in range(JT)]
                oss = [sb3.tile([P, FO], FP32, name=f"ost{i}", tag=f"ost{i}")
                       for i in range(JT)]

                def tpv(i):
                    b, m = i // 4, i % 4
                    return tp4[b][:, m * (FO + 1):(m + 1) * (FO + 1)]

                for i in range(JT):
                    nc.tensor.transpose(tpv(i), ht[:, i * P:(i + 1) * P],
                                        idn[0:FO + 1, 0:FO + 1])
                # gather the 32 denominators, invert in ONE op (DVE
                # reciprocal is expensive per-instruction on HW)
                for i in range(JT):
                    nc.vector.reciprocal(rcs[i][:], tpv(i)[:, FO:FO + 1])
                for i in range(JT):
                    nc.scalar.activation(eys[i][:], tpv(i)[:, 0:FO], AF.Exp,
                                         scale=rcs[i][:])
                for i in range(JT):
                    nc.vector.tensor_scalar(ems[i][:], eys[i][:],
                                            1.0, -1.0, ALU.min, ALU.add)
                for i in range(JT):
                    nc.vector.scalar_tensor_tensor(oss[i][:],
                                                   tpv(i)[:, 0:FO],
                                                   rcs[i][:],
                                                   ems[i][:],
                                                   ALU.mult, ALU.max)
                for i in range(JT):
                    nc.sync.dma_start(d_out[i * P:(i + 1) * P, :],
                                      oss[i][:])

    nc.compile()
    return nc


_NC_CACHE = None


def get_nc():
    global _NC_CACHE
    if _NC_CACHE is None:
        _NC_CACHE = build_nc()
    return _NC_CACHE


def make_in_maps(features, adjacency_matrix, W, a_src, a_dst):
    featT = np.ascontiguousarray(features.T).astype(ml_dtypes.bfloat16)
    adjT = np.ascontiguousarray(adjacency_matrix.T)
    dve_rows = np.concatenate(
        [np.arange(j * P, (j + 1) * P) for j in MULT_DVE])
    pool_tiles = [j for j in range(JT) if j not in DVE_SLOT]
    pool_rows = np.concatenate(
        [np.arange(j * P, (j + 1) * P) for j in pool_tiles])
    adjT8 = np.ascontiguousarray(adjT[pool_rows]).astype(
        ml_dtypes.float8_e4m3fn).view(np.uint8)
    adjTb = np.ascontiguousarray(adjT[dve_rows]).astype(ml_dtypes.bfloat16)
    ident = np.eye(P, dtype=np.float32)
    in_maps = []
    for h in range(C):
        in_maps.append({
            "featT": featT,
            "adjT8": adjT8,
            "adjTb": adjTb,
            "Wh": np.ascontiguousarray(W[h]).astype(ml_dtypes.bfloat16),
            "ah": np.ascontiguousarray(
                np.stack([a_src[h], a_dst[h]])).astype(ml_dtypes.bfloat16),
            "ident": ident,
        })
    return in_maps


def kernel(features, adjacency_matrix, W, a_src, a_dst, _trace=False, _tmpdir=None):
    nc = get_nc()
    in_maps = make_in_maps(np.asarray(features, dtype=np.float32),
                           np.asarray(adjacency_matrix),
                           np.asarray(W, dtype=np.float32),
                           np.asarray(a_src, dtype=np.float32),
                           np.asarray(a_dst, dtype=np.float32))
    res = run_bass_kernel_spmd(nc, in_maps, list(range(C)),
                               trace=_trace, tmpdir=_tmpdir)
    out = np.concatenate([res.results[h]["out"] for h in range(C)], axis=1)
    if _trace:
        kernel.last_results = res
    return out


# revision 31
# speedup vs baseline: 1.1589x; 1.1589x over previous
"""Multi-head graph attention (GAT) on 8 TRN2 NeuronCores.

Reference computation (N=4096 nodes, F_in=512, H=8 heads, F_out=64):
    Wh   = einsum('nf,hfo->hno', features, W)
    src  = Wh @ a_src  (per head), dst = Wh @ a_dst
    e    = leaky_relu(src_i + dst_j, 0.2), masked by adjacency
    attn = softmax(e, axis=-1)
    h    = elu(attn @ Wh)  -> concat heads -> [N, H*F_out]

Sharding: head parallelism - core c owns head c entirely. Each core computes
its head's Wh, the full [N, N] masked softmax, and output columns
out[:, 64c:64c+64]; the host gather is a concatenate. No collectives.

Per-core algorithm ("keys on partitions", scores transposed):
    exp(prelu(src_i + dst_j)) = max(exp(z), exp(0.2 z))        [z = s_i + d_j]
                              = F1[j] * E2[i] * max(E4[i], C0[j])
with E4 = exp(0.8 src), E2 = exp(0.2 src), F1 = exp(dst), C0 = exp(-0.8 dst).
The E2[i] factor is constant per query column, so it scales softmax numerator
and denominator identically and CANCELS - dropped entirely. F1[j] rides the
contraction index, so it folds into the matmul weights (including the ones
column that accumulates the denominator):
    acc[o,i] = sum_j (Wh[j,o]*F1[j]) * pm[j,i]
    pm[j,i]  = adj[j,i] * max(E4[i], C0[j])
pm is ONE fused scalar_tensor_tensor op, (e4rep max C0) * adj, split across
Pool and DVE; ACT only computes the O(N) exp vectors and the epilogue.
Adjacency streams as fp8 ({0,1} exact) to halve HBM traffic. Normalization +
ELU run on transposed [P, 65] tiles at the end.
"""
import numpy as np
import ml_dtypes

import concourse.bass as bass
import concourse.bacc as bacc
import concourse.tile as tile
import concourse.mybir as mybir
from concourse.bass_utils import run_bass_kernel_spmd

FP32 = mybir.dt.float32
BF16 = mybir.dt.bfloat16
FP8 = mybir.dt.float8e4
AF = mybir.ActivationFunctionType
ALU = mybir.AluOpType
AX = mybir.AxisListType

P = 128          # SBUF partitions
N = 4096         # nodes
F = 512          # input features
H = 8            # heads
FO = 64          # out features per head
C = 8            # cores (1 head each)
JT = N // P      # key tiles = 32
FC = F // P      # feature chunks = 4
QC = N // 512    # query column chunks of 512 = 8
ALPHA = 0.2

# --- tuning knobs (HW-microbenched: DVE ts 4x=1.12us, DVE tt bf16 2x=2.21us,
# Pool tt=7.7us at gpsimd eff 0.42, DVE stt 1x=4.35us, ACT pass=3.5us) ---
# Every tile's max-job (v = e4rep max C0) is a 4x tensor_scalar on DVE.
# The mask-mult splits: MULT_DVE tiles do a 2x bf16 tensor_tensor on DVE
# (bf16 adjacency); the rest multiply on Pool (fp8 adjacency - Pool tt is
# dtype-agnostic, and gpsimd supports only tensor_tensor/copy on HW).
POOL_TILES = (0, 3, 6, 9, 12, 15, 18, 21, 24, 26)
MULT_DVE = tuple(j for j in range(32) if j not in POOL_TILES)
DVE_SLOT = {j: k for k, j in enumerate(MULT_DVE)}


def build_nc(iters=1, loop_n=None, upto=3):
    nc = bacc.Bacc("TRN2", target_bir_lowering=False, debug=False)

    d_ft = nc.dram_tensor("featT", [F, N], BF16, kind="ExternalInput")
    n_dve = len(MULT_DVE)
    # uint8 at the XLA boundary (fp8 unsupported there on TRN2); the DMA
    # bitcasts to fp8e4m3 - same bytes
    d_adj8 = nc.dram_tensor("adjT8", [N - n_dve * P, N], mybir.dt.uint8,
                            kind="ExternalInput")
    d_adjb = nc.dram_tensor("adjTb", [n_dve * P, N], BF16,
                            kind="ExternalInput")
    d_wh = nc.dram_tensor("Wh", [F, FO], BF16, kind="ExternalInput")
    d_ah = nc.dram_tensor("ah", [2, FO], BF16, kind="ExternalInput")
    d_id = nc.dram_tensor("ident", [P, P], FP32, kind="ExternalInput")
    d_out = nc.dram_tensor("out", [N, FO], FP32, kind="ExternalOutput")

    from contextlib import ExitStack, nullcontext

    with tile.TileContext(nc) as tc:
      with (tc.For_i(0, loop_n, 1) if loop_n else nullcontext()):
       for _it in range(iters):
        with ExitStack() as stk:
            keep = stk.enter_context(tc.tile_pool(name="keep", bufs=1))

            # ---- persistent tiles ----
            e4rep = keep.tile([P, N], BF16)        # exp(0.8 src) replicated
            wf1 = [keep.tile([P, FO + 1], BF16, name=f"wf1{j}", tag=f"wf1{j}")
                   for j in range(JT)]
            sdg = [keep.tile([P, 8], FP32, name=f"sdg{g}", tag=f"sdg{g}")
                   for g in range(4)]              # dst proj per group of 8
            f1g = [keep.tile([P, 8], FP32, name=f"f1g{g}", tag=f"f1g{g}")
                   for g in range(4)]              # exp(dst)
            c0g = [keep.tile([P, 8], FP32, name=f"c0g{g}", tag=f"c0g{g}")
                   for g in range(4)]              # exp(-0.8 dst)
            idn = keep.tile([P, P], FP32)
            ones1 = keep.tile([1, P], BF16)
            ar = keep.tile([1, 2 * FO], BF16)
            arep = keep.tile([P, 2 * FO], BF16)
            wt = keep.tile([P, 2 * FC], BF16)      # col 2c = src, 2c+1 = dst
            ht = keep.tile([FO + 1, N], FP32)      # evacuated accumulator

            nc.scalar.dma_start(idn[:], d_id[:])
            nc.scalar.dma_start(ar[:], d_ah.ap().rearrange("(x s) o -> x (s o)", x=1))
            nc.vector.memset(ones1[:], 1.0)

            with ExitStack() as ph1:
                sb1 = ph1.enter_context(tc.tile_pool(name="sb1", bufs=1))
                ps1 = ph1.enter_context(tc.tile_pool(name="ps1", bufs=2, space="PSUM"))

                ft = sb1.tile([P, FC * N], BF16)         # featT, 32KB/part
                whsd = sb1.tile([P, FC * (FO + 1)], BF16)  # [Wh_c | wt_dst_c]
                # small weights first so wt/psr/pwh aren't stuck behind the
                # big feature loads; ft halves split across the two hwdge
                # queues (h0 -> ACT, h1 -> SP) so both finish by ~13us
                nc.scalar.dma_start(
                    whsd[:].rearrange("p (c o) -> p c o", c=FC)[:, :, 0:FO],
                    d_wh.ap().rearrange("(c p) o -> p c o", p=P))
                for c_ in range(FC):
                    nc.scalar.dma_start(
                        ft[:, c_ * N: c_ * N + 2048],
                        d_ft[c_ * P:(c_ + 1) * P, 0:2048])
                for c_ in range(FC):
                    nc.sync.dma_start(
                        ft[:, c_ * N + 2048: c_ * N + 4096],
                        d_ft[c_ * P:(c_ + 1) * P, 2048:4096])

                # broadcast [a_src | a_dst] across partitions (k=1 matmul)
                ps_b = ps1.tile([P, 2 * FO], FP32, tag="bc", bufs=1)
                nc.tensor.matmul(ps_b[:], ones1[:], ar[:], start=True, stop=True)
                nc.vector.tensor_copy(arep[:], ps_b[:])

                # wtilde[f] = sum_o Wh[f, o] * a[o]  (src col -> wt, dst col ->
                # wt and whsd's 65th column per chunk)
                lp = stk.enter_context(
                    nc.allow_low_precision(reason="bf16 projection weights"))
                for c_ in range(FC):
                    tmp = sb1.tile([P, 2 * FO], BF16, tag="wtmp")
                    nc.vector.tensor_tensor(
                        tmp[:, 0:FO],
                        whsd[:, c_ * (FO + 1):c_ * (FO + 1) + FO],
                        arep[:, 0:FO], ALU.mult)
                    nc.vector.tensor_tensor(
                        tmp[:, FO:2 * FO],
                        whsd[:, c_ * (FO + 1):c_ * (FO + 1) + FO],
                        arep[:, FO:2 * FO], ALU.mult)
                    nc.vector.tensor_reduce(wt[:, 2 * c_:2 * c_ + 2],
                                            tmp[:].rearrange("p (s o) -> p s o", s=2),
                                            AX.X, ALU.add)
                    nc.vector.tensor_copy(whsd[:, c_ * (FO + 1) + FO:(c_ + 1) * (FO + 1)],
                                          wt[:, 2 * c_ + 1:2 * c_ + 2])

                # stage-major: all psr matmuls first, then all exps, then
                # all broadcasts - engine queues pipeline independently, so
                # the per-chunk PE<->ACT round-trips disappear.
                er4all = sb1.tile([1, N], BF16)
                psrs = {}

                def emit_psr(q):
                    psr = ps1.tile([1, 512], FP32, name=f"psr{q}",
                                   tag="psr", bufs=2)
                    for c_ in range(FC):
                        nc.tensor.matmul(psr[:], wt[:, 2 * c_:2 * c_ + 1],
                                         ft[:, c_ * N + q * 512:c_ * N + (q + 1) * 512],
                                         start=(c_ == 0), stop=(c_ == FC - 1))
                    psrs[q] = psr

                def emit_exp(q):
                    nc.scalar.activation(er4all[:, q * 512:(q + 1) * 512],
                                         psrs[q][:], AF.Exp, scale=0.8)

                def emit_bcast(q):
                    pb = ps1.tile([P, 512], FP32, tag="pb", bufs=1)
                    nc.tensor.matmul(pb[:], ones1[:],
                                     er4all[:, q * 512:(q + 1) * 512],
                                     start=True, stop=True)
                    nc.scalar.copy(e4rep[:, q * 512:(q + 1) * 512], pb[:])

                whd_tiles = {}

                def emit_group(g):
                    # 8 tiles: fused Wh+dst matmuls, then exp packs, then Wf1
                    for j in range(8 * g, 8 * g + 8):
                        pw = ps1.tile([P, FO + 1], FP32, name=f"pw{j}",
                                      tag="pw", bufs=4)
                        for c_ in range(FC):
                            nc.tensor.matmul(pw[:], ft[:, c_ * N + j * P: c_ * N + j * P + P],
                                             whsd[:, c_ * (FO + 1):(c_ + 1) * (FO + 1)],
                                             start=(c_ == 0), stop=(c_ == FC - 1))
                        whd = sb1.tile([P, FO], BF16, name=f"whd{j}",
                                       tag=f"whd{j % 16}")
                        nc.scalar.copy(whd[:], pw[:, 0:FO])
                        nc.scalar.copy(sdg[g][:, j % 8:j % 8 + 1],
                                       pw[:, FO:FO + 1])
                        whd_tiles[j] = whd
                    nc.scalar.activation(f1g[g][:], sdg[g][:], AF.Exp)
                    nc.scalar.activation(c0g[g][:], sdg[g][:], AF.Exp,
                                         scale=-0.8)
                    for j in range(8 * g, 8 * g + 8):
                        nc.vector.tensor_scalar(wf1[j][:, 0:FO], whd_tiles[j][:],
                                                f1g[g][:, j % 8:j % 8 + 1],
                                                None, ALU.mult)
                        nc.scalar.copy(wf1[j][:, FO:FO + 1],
                                       f1g[g][:, j % 8:j % 8 + 1])

                for q in range(QC):
                    emit_psr(q)
                for q in range(QC):
                    emit_exp(q)
                for q in range(QC):
                    emit_bcast(q)
                for g in range(4):
                    emit_group(g)

            if upto < 1:
                with ExitStack() as phx:
                    sbx = phx.enter_context(tc.tile_pool(name="sbx", bufs=1))
                    junk = sbx.tile([P, FO], FP32)
                    nc.vector.memset(junk[:], 0.0)
                    for i in range(JT):
                        nc.sync.dma_start(d_out[i * P:(i + 1) * P, :], junk[:])
                continue
            # ---- phase 2: fused masked-exp scores + V-matmul ----
            sb2 = stk.enter_context(tc.tile_pool(name="sb2", bufs=2))
            adjp = stk.enter_context(tc.tile_pool(name="adjp", bufs=4))
            with ExitStack() as ph2:
                acc_pool = ph2.enter_context(
                    tc.tile_pool(name="accps", bufs=1, space="PSUM"))
                acc = acc_pool.tile([FO + 1, N], FP32)   # all 8 banks

                pool_slot = 0
                HN = N // 2
                for j in range(JT):
                    g, s = j // 8, j % 8
                    v = sb2.tile([P, N], BF16, tag="v", bufs=4)
                    # pm = (e4rep max C0[j]) * adj
                    nc.vector.tensor_scalar(v[:], e4rep[:],
                                            c0g[g][:, s:s + 1], None, ALU.max)
                    if j in DVE_SLOT:
                        k = DVE_SLOT[j]
                        at = adjp.tile([P, N], BF16, tag="atb", bufs=4)
                        dq = nc.scalar if k % 2 == 0 else nc.sync
                        dq.dma_start(at[:], d_adjb[k * P:(k + 1) * P, :])
                        pm = sb2.tile([P, N], BF16, tag="pm", bufs=4)
                        nc.vector.tensor_tensor(pm[:], at[:], v[:], ALU.mult)
                        halves = [pm[:, 0:HN], pm[:, HN:N]]
                    else:
                        k = pool_slot
                        pool_slot += 1
                        at = adjp.tile([P, N], FP8, tag="at", bufs=4)
                        nc.sync.dma_start(at[:], d_adj8[k * P:(k + 1) * P, :].bitcast(FP8))
                        pma = sb2.tile([P, HN], BF16, tag="pma", bufs=3)
                        pmb = sb2.tile([P, HN], BF16, tag="pmb", bufs=3)
                        nc.gpsimd.tensor_tensor(pma[:], at[:, 0:HN],
                                                v[:, 0:HN], ALU.mult)
                        nc.gpsimd.tensor_tensor(pmb[:], at[:, HN:N],
                                                v[:, HN:N], ALU.mult)
                        halves = [pma[:], pmb[:]]
                    for q in range(QC):
                        h = halves[q // 4]
                        nc.tensor.matmul(acc[:, q * 512:(q + 1) * 512], wf1[j][:],
                                         h[:, (q % 4) * 512:(q % 4 + 1) * 512],
                                         start=(j == 0), stop=(j == JT - 1))

                for q in range(4):
                    nc.scalar.copy(ht[:, q * 1024:(q + 1) * 1024],
                                   acc[:, q * 1024:(q + 1) * 1024])

            if upto < 2:
                with ExitStack() as phx:
                    sbx = phx.enter_context(tc.tile_pool(name="sbx", bufs=1))
                    junk = sbx.tile([P, FO], FP32)
                    nc.vector.memset(junk[:], 0.0)
                    for i in range(JT):
                        nc.sync.dma_start(d_out[i * P:(i + 1) * P, :], junk[:])
                continue
            # ---- epilogue: transpose, normalize, ELU ----
            # y = tp * rcol;  elu(y) = max(y, min(exp(y),1) - 1)
            # Stage-major emission: each engine streams 32 independent ops
            # per stage instead of ping-ponging cross-engine per tile
            # (in-order queues turn interleaved chains into serial
            # round-trips on HW). tp tiles pack 4 transposes per PSUM bank.
            with ExitStack() as ph3:
                ps3 = ph3.enter_context(tc.tile_pool(name="ps3", bufs=1, space="PSUM"))
                sb3 = ph3.enter_context(tc.tile_pool(name="sb3", bufs=1))
                tp4 = [ps3.tile([P, 4 * (FO + 1)], FP32, name=f"tp4{b}",
                                tag=f"tp4{b}") for b in range(8)]
                rcs = [sb3.tile([P, 1], FP32, name=f"rc{i}", tag=f"rc{i}")
                       for i in range(JT)]
                eys = [sb3.tile([P, FO], FP32, name=f"eyt{i}", tag=f"eyt{i}")
                       for i in range(JT)]
                ems = [sb3.tile([P, FO], FP32, name=f"emt{i}", tag=f"emt{i}")
                       for i in range(JT)]
                oss = [sb3.tile([P, FO], FP32, name=f"ost{i}", tag=f"ost{i}")
                       for i in range(JT)]

                def tpv(i):
                    b, m = i // 4, i % 4
                    return tp4[b][:, m * (FO + 1):(m + 1) * (FO + 1)]

                for i in range(JT):
                    nc.tensor.transpose(tpv(i), ht[:, i * P:(i + 1) * P],
                                        idn[0:FO + 1, 0:FO + 1])
                # gather the 32 denominators, invert in ONE op (DVE
                # reciprocal is expensive per-instruction on HW)
                for i in range(JT):
                    nc.vector.reciprocal(rcs[i][:], tpv(i)[:, FO:FO + 1])
                for i in range(JT):
                    nc.scalar.activation(eys[i][:], tpv(i)[:, 0:FO], AF.Exp,
                                         scale=rcs[i][:])
                for i in range(JT):
                    nc.vector.tensor_scalar(ems[i][:], eys[i][:],
                                            1.0, -1.0, ALU.min, ALU.add)
                for i in range(JT):
                    nc.vector.scalar_tensor_tensor(oss[i][:],
                                                   tpv(i)[:, 0:FO],
                                                   rcs[i][:],
                                                   ems[i][:],
                                                   ALU.mult, ALU.max)
                for i in range(JT):
                    nc.sync.dma_start(d_out[i * P:(i + 1) * P, :],
                                      oss[i][:])

    nc.compile()
    return nc


_NC_CACHE = None


def get_nc():
    global _NC_CACHE
    if _NC_CACHE is None:
        _NC_CACHE = build_nc()
    return _NC_CACHE


def make_in_maps(features, adjacency_matrix, W, a_src, a_dst):
    featT = np.ascontiguousarray(features.T).astype(ml_dtypes.bfloat16)
    adjT = np.ascontiguousarray(adjacency_matrix.T)
    dve_rows = np.concatenate(
        [np.arange(j * P, (j + 1) * P) for j in MULT_DVE])
    pool_tiles = [j for j in range(JT) if j not in DVE_SLOT]
    pool_rows = np.concatenate(
        [np.arange(j * P, (j + 1) * P) for j in pool_tiles])
    adjT8 = np.ascontiguousarray(adjT[pool_rows]).astype(
        ml_dtypes.float8_e4m3fn).view(np.uint8)
    adjTb = np.ascontiguousarray(adjT[dve_rows]).astype(ml_dtypes.bfloat16)
    ident = np.eye(P, dtype=np.float32)
    in_maps = []
    for h in range(C):
        in_maps.append({
            "featT": featT,
            "adjT8": adjT8,
            "adjTb": adjTb,
            "Wh": np.ascontiguousarray(W[h]).astype(ml_dtypes.bfloat16),
            "ah": np.ascontiguousarray(
                np.stack([a_src[h], a_dst[h]])).astype(ml_dtypes.bfloat16),
            "ident": ident,
        })
    return in_maps


def kernel(features, adjacency_matrix, W, a_src, a_dst, _trace=False, _tmpdir=None):
    nc = get_nc()
    in_maps = make_in_maps(np.asarray(features, dtype=np.float32),
                           np.asarray(adjacency_matrix),
                           np.asarray(W, dtype=np.float32),
                           np.asarray(a_src, dtype=np.float32),
                           np.asarray(a_dst, dtype=np.float32))
    res = run_bass_kernel_spmd(nc, in_maps, list(range(C)),
                               trace=_trace, tmpdir=_tmpdir)
    out = np.concatenate([res.results[h]["out"] for h in range(C)], axis=1)
    if _trace:
        kernel.last_results = res
    return out
